# revision 26
# baseline (speedup 1.0000x reference)
"""NetVLAD Trainium2 kernel (data-parallel over batch across 8 NeuronCores).

Math per image (x: [D=512, P=4096], conv_w: [K=64, D], centroids c: [K, D]):
  xhat = x / ||x||_2(over D, per pixel)
  logitsT[p, k] = sum_d xhat[d, p] * conv_w[k, d]
  a = softmax_k(logitsT)            (|logits| <= ||w_k|| ~ 1.3 -> no max-sub)
  vlad[k, d] = sum_p a[p, k] * xhat[d, p] - (sum_p a[p, k]) * c[k, d]
  out = l2norm_global(l2norm_rows(vlad))

Folding: raw logits are computed from RAW x and the per-pixel
temperature is applied at exp time; xhatT is materialized directly by
scaling the PE-transposed x chunks with invnorm during the PSUM->SBUF
copy (per-partition scale, pixels on partitions):
  e[p,k]   = exp(invnorm[p] * raw_logit[p,k])
  xhatT    = (x^T chunk) * invnorm[p]          (fused into the copy)
  e2[p,k]  = e[p,k] / S[p]                     S = sum_k e
  vlad     = sum_p e2[p,k] * xhatT[p,d]        (matmul2)
  A[k]     = sum_p e2[p,k] * 1                 (2nd matmul, rhs=ones,
                                                shares lhsT=e2)

Engine plan. The ACT engine never switches activation tables (each
reload is 1.28us and was 24%% of the baseline runtime): only
{Copy, Exp, Square} are used - all in the exp_and_others table. No
Sqrt/Ln anywhere: invnorm = rsqrt(sumsq) via bit-trick seed + Newton
on [128,8] tiles (DVE); sumsq via x^2 (DVE/Pool) contracted by PE
ones-matmuls (output free size 1). The global-norm partition reduction
is a PE ones-matrix matmul (broadcast sum), not a gpsimd library call.

Latency plan (the critical path is the per-group softmax chain):
  - x is loaded per GROUP (2MB pieces, 6-deep pool) so image
    boundaries never wait on a full-image load.
  - The x^2 / ones-matmul / Newton pipeline for group-slot t+1 runs
    during slot t, so invnorm is ready before slot t+1's copies start.
  - matmul2 for each group is deferred one group-slot in a flat
    (image, group) pipeline crossing image boundaries; each image's
    epilogue runs right after its last deferred matmul2. matmul2
    accumulates all 32 chunks into one PSUM tile (start only on the
    first matmul, stop at each group end so the PE may re-enter
    transpose mode, start=False continues accumulation); the epilogue
    reads vlad straight from PSUM.

Implementation notes (bf16 on-chip via SWDGE cast-DMA; Bacc handles the
TRN2 one-wait-per-instruction split via generate_event_semaphores):
  - The PE hard-faults if it enters transpose mode while any PSUM
    accumulation group is open, so all accumulation groups (logits,
    ones, matmul2 per-group runs) close before transposes follow.
  - DMA transposes (xbar) are unusable here: the XPOSE struct takes only
    one sem wait and Tile cannot consolidate its multi-proc deps.
  - DMA has no PSUM route and the PE reads only SBUF, so the xT
    PSUM->SBUF copies must go through DVE/ACT (split tunable).
"""

import numpy as np
import ml_dtypes

N, D, HH, WW, K = 32, 512, 64, 64, 64
P = HH * WW            # 4096
NCORES = 8
NPER = N // NCORES     # 4 images per core
DC = D // 128          # 4 d-chunks
PC = P // 128          # 32 p-chunks
GRP = 8                # p-chunks per softmax batch group
NG = PC // GRP
PG = P // NG           # pixels per group (1024)

TRACE = False          # test.py sets this for profiled runs
IMGS = NPER            # debug knob: build fewer images
STAGE = 5              # kept for timeit3 API compat (unused)
REPS = 1               # timing knob: hardware-loop the whole body REPS times
DVE_COPY = 1           # xT pair-copies per group on DVE (rest ACT; 4 pairs/group)
POOL_XSQ = 2           # x^2 squarings per group-slot on Pool (rest DVE)
NEWTON_ITERS = 1       # Newton refinements of the bit-trick seed (main loop)
EP_POOL = 1            # e' multiply on Pool (else DVE)
LOOKAHEAD = 4          # x group-loads issued this many group-slots ahead
MAGIC = 0x5F3759DF
_CACHE = {}

def _build():
    import concourse.bass as bass
    import concourse.bacc as bacc
    import concourse.tile as tile
    from concourse import mybir
    import concourse.bass_isa as bass_isa
    from contextlib import nullcontext

    f32 = mybir.dt.float32
    bf16 = mybir.dt.bfloat16
    u32 = mybir.dt.uint32
    FT = mybir.ActivationFunctionType
    ALU = mybir.AluOpType

    nc = bacc.Bacc()
    x_dram = nc.declare_dram_parameter("x", [NPER, D, P], f32, isOutput=False)
    wt_dram = nc.declare_dram_parameter("conv_wt", [D, K], bf16, isOutput=False)
    id_dram = nc.declare_dram_parameter("ident", [128, 128], bf16, isOutput=False)
    c_dram = nc.declare_dram_parameter("cent", [K, D], f32, isOutput=False)
    y_dram = nc.declare_dram_parameter("y", [NPER, K * D], f32, isOutput=True)

    with tile.TileContext(nc) as tc:
        with (
            tc.tile_pool(name="consts", bufs=1) as consts,
            tc.tile_pool(name="xgrp", bufs=LOOKAHEAD + 2) as xgrp,
            tc.tile_pool(name="xsqp", bufs=2) as xsqp,
            tc.tile_pool(name="slot", bufs=4) as slotp,
            tc.tile_pool(name="xtpool", bufs=2) as xtpool,
            tc.tile_pool(name="epool", bufs=2) as epool,
            tc.tile_pool(name="stats", bufs=2) as stats,
            tc.tile_pool(name="dumps", bufs=1) as dumps,
            tc.tile_pool(name="epi", bufs=2) as epi,
            tc.tile_pool(name="ps_xt", bufs=2, space="PSUM") as ps_xt,
            tc.tile_pool(name="ps_log", bufs=2, space="PSUM") as ps_log,
            tc.tile_pool(name="ps_ss", bufs=1, space="PSUM") as ps_ss,
            tc.tile_pool(name="ps_small", bufs=1, space="PSUM") as ps_small,
            tc.tile_pool(name="ps_vlad", bufs=1, space="PSUM") as ps_vlad,
        ):
            # ---- constants ----
            wt_sb = consts.tile([128, DC, K], bf16)   # conv_w^T chunked
            nc.sync.dma_start(
                out=wt_sb[:], in_=wt_dram[:].rearrange("(dc dp) k -> dp dc k", dp=128)
            )
            ident = consts.tile([128, 128], bf16)
            nc.sync.dma_start(out=ident[:], in_=id_dram[:])
            cent_sb = consts.tile([K, D], f32)
            nc.sync.dma_start(out=cent_sb[:], in_=c_dram[:])
            ones = consts.tile([128, 1], bf16)
            nc.vector.memset(ones[:], 1.0)
            onesq = consts.tile([K, K], bf16)
            nc.vector.memset(onesq[:], 1.0)
            magic = consts.tile([128, 1], u32)
            nc.vector.memset(magic[:], MAGIC)

            epi_dump = dumps.tile([K, D], f32)

            def rsqrt_newton(
                eng, y, y_u, ss, ss_u, t_u, q, r, w, wd, iters=NEWTON_ITERS
            ):
                """y = rsqrt(ss) elementwise on [*, wd] APs via bit-trick
                seed + Newton (y *= 1.5 - 0.5*ss*y*y). All ops on `eng`.
                ss may live in PSUM; scratch and y must be SBUF."""
                mg = magic[0:ss.shape[0], 0:1].to_broadcast([ss.shape[0], wd])
                eng.tensor_scalar(
                    out=t_u, in0=ss_u, scalar1=1, scalar2=None,
                    op0=ALU.logical_shift_right,
                )
                eng.tensor_tensor(out=y_u, in0=mg, in1=t_u, op=ALU.subtract)
                for _ in range(iters):
                    eng.tensor_tensor(out=q, in0=y, in1=y, op=ALU.mult)
                    eng.tensor_tensor(out=r, in0=q, in1=ss, op=ALU.mult)
                    eng.tensor_scalar(
                        out=w, in0=r, scalar1=-0.5, scalar2=1.5,
                        op0=ALU.mult, op1=ALU.add,
                    )
                    eng.tensor_tensor(out=y, in0=y, in1=w, op=ALU.mult)

            loop_ctx = tc.For_i(0, REPS, 1) if REPS > 1 else nullcontext()
            with loop_ctx:
                steps = [(nn, g) for nn in range(IMGS) for g in range(NG)]
                xg_tiles = {}
                inv_tiles = {}
                img = {}       # per-image tile sets, keyed by nn
                pending = []   # steps whose matmul2 is not yet emitted

                def issue_load(t):
                    if t >= len(steps):
                        return
                    nn, g = steps[t]
                    n = nn % NPER
                    xg = xgrp.tile([128, DC, PG], bf16, tag="xg", name="xg")
                    src = x_dram[n, :, g * PG:(g + 1) * PG].rearrange(
                        "(dc dp) p -> dp dc p", dp=128
                    )
                    nc.gpsimd.dma_start(out=xg[:], in_=src[:])
                    xg_tiles[t] = xg

                def invnorm_phase(t):
                    """x^2, PE ones-contraction, and Newton rsqrt for
                    group-slot t (runs one slot early)."""
                    if t >= len(steps):
                        return
                    xg = xg_tiles[t]
                    xsq = xsqp.tile([128, DC, PG], bf16, tag="xsq", name="xsq")
                    split = 128 * (GRP - POOL_XSQ)
                    nc.vector.tensor_tensor(
                        out=xsq[:, :, 0:split], in0=xg[:, :, 0:split],
                        in1=xg[:, :, 0:split], op=ALU.mult,
                    )
                    nc.gpsimd.tensor_tensor(
                        out=xsq[:, :, split:PG], in0=xg[:, :, split:PG],
                        in1=xg[:, :, split:PG], op=ALU.mult,
                    )
                    ss = ps_ss.tile([128, GRP], f32, tag="ss", name="ss")
                    for sub in range(GRP):
                        lcs = slice(sub * 128, (sub + 1) * 128)
                        for dc in range(DC):
                            nc.tensor.matmul(
                                ss[:, sub:sub + 1],
                                lhsT=xsq[:, dc, lcs],
                                rhs=ones[:],
                                start=(dc == 0),
                                stop=(dc == DC - 1),
                            )
                    inv = slotp.tile([128, GRP, 1], f32, tag="inv", name="inv")
                    t_u = slotp.tile([128, GRP], u32, tag="t_u", name="t_u")
                    nq = slotp.tile([128, GRP], f32, tag="nq", name="nq")
                    nr = slotp.tile([128, GRP], f32, tag="nr", name="nr")
                    nw = slotp.tile([128, GRP], f32, tag="nw", name="nw")
                    rsqrt_newton(
                        nc.vector,
                        y=inv[:, :, 0], y_u=inv[:, :, 0].bitcast(u32),
                        ss=ss[:], ss_u=ss[:].bitcast(u32),
                        t_u=t_u[:], q=nq[:], r=nr[:], w=nw[:], wd=GRP,
                    )
                    norm = slotp.tile([128, GRP], bf16, tag="norm", name="norm")
                    nc.vector.tensor_tensor(
                        out=norm[:], in0=ss[:], in1=inv[:, :, 0], op=ALU.mult
                    )
                    inv_tiles[t] = (inv, norm)

                def alloc_image(nn):
                    T = {}
                    T["e"] = epool.tile([128, PC, K], bf16, tag="e", name="e")
                    T["slog"] = epool.tile(
                        [128, PC, K], bf16, tag="slog", name="slog"
                    )
                    T["ep"] = epool.tile([128, PC, K], bf16, tag="ep", name="ep")
                    T["S"] = stats.tile([128, NG, GRP], bf16, tag="S", name="S")
                    T["invS"] = stats.tile(
                        [128, PC, 1], f32, tag="invS", name="invS"
                    )
                    T["factor"] = stats.tile(
                        [128, PC, 1], bf16, tag="factor", name="factor"
                    )
                    T["xt"] = xtpool.tile(
                        [128, PC, 512], bf16, tag="xt", name="xt"
                    )
                    T["vlad_ps"] = ps_vlad.tile(
                        [K, D], f32, tag="vlad_ps", name="vlad_ps"
                    )
                    T["a_ps"] = ps_small.tile(
                        [K, 1], f32, tag="a_ps", name="a_ps"
                    )
                    return T

                def mm2_group(T, gg, norm):
                    # wait-absorbing weights-load (also warms the PE stream)
                    nc.tensor.ldweights(weights=T["ep"][:, gg * GRP + 2])
                    for sub in range(GRP):
                        pc = gg * GRP + sub
                        nc.tensor.matmul(
                            T["vlad_ps"][:],
                            lhsT=T["ep"][:, pc],
                            rhs=T["xt"][:, pc],
                            start=(pc == 0),
                            stop=(sub == GRP - 1),
                            skip_group_check=True,
                        )
                        nc.tensor.matmul(
                            T["a_ps"][:],
                            lhsT=T["ep"][:, pc],
                            rhs=norm[:, sub:sub + 1],
                            start=(pc == 0),
                            stop=(sub == GRP - 1),
                            skip_group_check=True,
                        )

                def epilogue(T, n):
                    a_sb = epi.tile([K, 1], f32)
                    nc.vector.tensor_copy(a_sb[:], T["a_ps"][:])
                    # vlad_neg = cent*A - vlad (sign fixed by negating the
                    # final scale; the Square accumulations are sign-blind)
                    vlad_neg = epi.tile([K, D], f32)
                    nc.vector.scalar_tensor_tensor(
                        out=vlad_neg[:], in0=cent_sb[:], scalar=a_sb[:],
                        in1=T["vlad_ps"][:], op0=ALU.mult, op1=ALU.subtract,
                    )
                    # intra (row) l2 norm via Square-accum + Newton rsqrt
                    rss = epi.tile([K, 1], f32)
                    nc.scalar.activation(
                        epi_dump[:], vlad_neg[:], FT.Square, accum_out=rss[:]
                    )
                    rinv = epi.tile([K, 1], f32)
                    et_u = epi.tile([K, 1], u32)
                    eq = epi.tile([K, 1], f32)
                    er = epi.tile([K, 1], f32)
                    ew = epi.tile([K, 1], f32)
                    rsqrt_newton(
                        nc.vector,
                        y=rinv[:], y_u=rinv[:].bitcast(u32),
                        ss=rss[:], ss_u=rss[:].bitcast(u32),
                        t_u=et_u[:], q=eq[:], r=er[:], w=ew[:], wd=1,
                        iters=2,
                    )
                    # global l2 norm: Square(vlad_neg * rinv)-accum without
                    # materializing the normalized rows, then a PE
                    # ones-matrix matmul broadcast-sums across partitions
                    gss = epi.tile([K, 1], f32)
                    nc.scalar.activation(
                        epi_dump[:], vlad_neg[:], FT.Square, scale=rinv[:],
                        accum_out=gss[:],
                    )
                    gss_bf = epi.tile([K, 1], bf16)
                    nc.vector.tensor_copy(gss_bf[:], gss[:])
                    gtot = ps_small.tile([K, 1], f32, tag="g_ps", name="g_ps")
                    nc.tensor.matmul(
                        gtot[:], lhsT=onesq[:], rhs=gss_bf[:],
                        start=True, stop=True,
                    )
                    ginv = epi.tile([K, 1], f32)
                    rsqrt_newton(
                        nc.vector,
                        y=ginv[:], y_u=ginv[:].bitcast(u32),
                        ss=gtot[:], ss_u=gtot[:].bitcast(u32),
                        t_u=et_u[:], q=eq[:], r=er[:], w=ew[:], wd=1,
                        iters=2,
                    )
                    # y = vlad_neg * (-rinv*ginv)
                    rg = epi.tile([K, 1], f32)
                    nc.vector.tensor_scalar(
                        out=rg[:], in0=rinv[:], scalar1=ginv[:], scalar2=-1.0,
                        op0=ALU.mult, op1=ALU.mult,
                    )
                    y_sb = epi.tile([K, D], f32)
                    nc.scalar.activation(
                        y_sb[:], vlad_neg[:], FT.Copy, scale=rg[:]
                    )
                    nc.sync.dma_start(
                        out=y_dram[n].rearrange("(k d) -> k d", d=D), in_=y_sb[:]
                    )

                for t in range(LOOKAHEAD):
                    issue_load(t)
                invnorm_phase(0)

                for t, (nn, g) in enumerate(steps):
                    issue_load(t + LOOKAHEAD)
                    if g == 0:
                        img[nn] = alloc_image(nn)
                        img.pop(nn - 2, None)
                    T = img[nn]
                    xg = xg_tiles.pop(t)
                    inv, norm = inv_tiles.pop(t)

                    # invnorm pipeline for the NEXT slot (x^2 + PE ones
                    # + Newton) - runs while this slot's chunks proceed
                    invnorm_phase(t + 1)

                    # PE wait-absorber for the group's x-load wait
                    nc.tensor.ldweights(weights=xg[:, 0, 0:128])

                    logT = ps_log.tile([128, GRP, K], f32, tag="logT", name="logT")
                    xtp = None
                    for sub in range(GRP):
                        pc = g * GRP + sub
                        lcs = slice(sub * 128, (sub + 1) * 128)
                        half = sub % 2
                        if half == 0:
                            xtp = ps_xt.tile(
                                [128, 2, 512], bf16, tag="xtp", name="xtp"
                            )
                        # ---- PE: logits + transpose ----
                        for dc in range(DC):
                            nc.tensor.matmul(
                                logT[:, sub],
                                lhsT=xg[:, dc, lcs],
                                rhs=wt_sb[:, dc],
                                start=(dc == 0),
                                stop=(dc == DC - 1),
                            )
                        for dc in range(DC):
                            nc.tensor.transpose(
                                xtp[:, half, dc * 128:(dc + 1) * 128],
                                xg[:, dc, lcs], ident[:]
                            )
                        # ---- xT pair copy PSUM -> SBUF (DVE/ACT split) ----
                        if half == 1:
                            pr = sub // 2
                            dst = T["xt"][:, pc - 1:pc + 1]
                            if pr < DVE_COPY:
                                nc.vector.tensor_copy(dst, xtp[:])
                            else:
                                nc.scalar.activation(dst, xtp[:], FT.Copy)

                    # ---- softmax for this group ----
                    gs = slice(g * GRP, (g + 1) * GRP)
                    # slog = logT * invnorm (broadcast over K), PSUM -> bf16
                    nc.vector.tensor_tensor(
                        out=T["slog"][:, gs],
                        in0=logT[:],
                        in1=inv[:].to_broadcast([128, GRP, K]),
                        op=ALU.mult,
                    )
                    # e = exp(slog): one batched ACT op per group
                    nc.scalar.activation(T["e"][:, gs], T["slog"][:, gs], FT.Exp)
                    with nc.allow_low_precision(
                        reason="softmax denom in bf16; small uniform scale "
                        "noise averages out across 4096 pixels"
                    ):
                        nc.vector.reduce_sum(
                            T["S"][:, g], T["e"][:, gs],
                            axis=mybir.AxisListType.X,
                        )
                    nc.vector.reciprocal(T["invS"][:, gs, 0], T["S"][:, g])
                    nc.vector.tensor_tensor(
                        out=T["factor"][:, gs, 0], in0=T["invS"][:, gs, 0],
                        in1=inv[:, :, 0], op=ALU.mult,
                    )
                    ep_eng = nc.gpsimd if EP_POOL else nc.vector
                    ep_eng.tensor_tensor(
                        out=T["ep"][:, gs],
                        in0=T["e"][:, gs],
                        in1=T["factor"][:, gs].to_broadcast([128, GRP, K]),
                        op=ALU.mult,
                    )
                    # deferred matmul2: one group-slot behind, crossing
                    # image boundaries so the PE never waits on the tail
                    pending.append((nn, g, norm))
                    if len(pending) > 1:
                        nn2, g2, norm2 = pending.pop(0)
                        mm2_group(img[nn2], g2, norm2)
                        if g2 == NG - 1:
                            epilogue(img[nn2], nn2 % NPER)
                while pending:
                    nn2, g2, norm2 = pending.pop(0)
                    mm2_group(img[nn2], g2, norm2)
                    if g2 == NG - 1:
                        epilogue(img[nn2], nn2 % NPER)
    nc.compile()
    return nc


def _get_nc():
    if "nc" not in _CACHE:
        _CACHE["nc"] = _build()
    return _CACHE["nc"]


def _in_maps(x, conv_w, centroids):
    xs = np.ascontiguousarray(x.reshape(NCORES, NPER, D, P))
    wt = np.ascontiguousarray(conv_w.T).astype(ml_dtypes.bfloat16)
    ident = np.eye(128, dtype=ml_dtypes.bfloat16)
    cent = np.ascontiguousarray(centroids).astype(np.float32)
    return [
        {"x": xs[c], "conv_wt": wt, "ident": ident, "cent": cent}
        for c in range(NCORES)
    ]


def kernel(x, conv_w, centroids):
    from concourse.bass_utils import run_bass_kernel_spmd

    nc = _get_nc()
    in_maps = _in_maps(x, conv_w, centroids)
    res = run_bass_kernel_spmd(
        nc, in_maps, core_ids=list(range(NCORES)), trace=TRACE
    )
    _CACHE["last_result"] = res
    y = np.concatenate([r["y"] for r in res.results], axis=0)
    return y.reshape(N, K * D)
